# revision 2
# baseline (speedup 1.0000x reference)
"""Trainium2 Bass kernel for nn_AMIML_5102421148136 (topk_masking), v2.

Data-parallel over B=8 across 8 cores. Per core:
  Phase A: x loaded in 28 groups of 3584 rows ([128 part, 28 rows] contiguous,
    SWDGE f32->fp8 cast, 28KB reads / 7KB writes per partition). Pair-packed
    PE transposes, fp8 DoubleRow L1, bf16 L2/L3, score matmul emits +s and -s
    rows; scores staged to SBUF score tiles s_pos/s_neg [112, 896].
  Selection (no gpsimd topk): per-partition top-16 via 2 rounds of
    max8/find_index8/match_replace (halves overlapped with phase A);
    theta = 104th-largest of the 112 partition maxes (exact superset bound);
    survivors counted, prefix-summed via triangular matmul, compacted into
    compact_d by one multi-index indirect scatter per sign (OOB-masked pads).
  Phase B: gather candidate x rows, exact fp32 refine MLP on 1024 cols,
    masked 13-round exact top-104 select per sign, 200-token attention,
    final softmax. Host stacks the 8 per-core rows.
"""

import sys
import math

if '/opt/trn_rl_repo' not in sys.path:
    sys.path.insert(0, '/opt/trn_rl_repo')

import numpy as np
import ml_dtypes

import concourse.bass as bass
import concourse.mybir as mybir
from concourse.tile import TileContext
from concourse.tile_rust import add_dep_helper
from concourse import bass_utils, bacc

F32 = mybir.dt.float32
BF16 = mybir.dt.bfloat16
F8 = mybir.dt.float8e4
U32 = mybir.dt.uint32
AF = mybir.ActivationFunctionType
ALU = mybir.AluOpType
DR = mybir.MatmulPerfMode.DoubleRow
AX = mybir.AxisListType

# ---- problem constants ----
B = 8
N = 100000
C = 256
D = 8
R = 100
ALPHA = 0.1
EPS = 1e-5
GRP = 3584             # positions per group (28 rows x 128 partitions)
NG = 28                # groups; NG*GRP = 100352 = NPAD
NPAD = NG * GRP
SP = 112               # score partitions per sign; SP*896 = NPAD
CAP = 512              # candidate capacity per sign
NEG = -1.0e30
NEGP = -1.0e6          # pad score
NEGR = -1.0e9          # replace/mask value
TOK = 256

_CACHE = {}
LAST = {}


def _fold_weights(inputs):
    w = {}

    def fold(li):
        cw = inputs[f'conv{li}_w'].astype(np.float64)
        cb = inputs[f'conv{li}_b'].astype(np.float64)
        g = inputs[f'bn{li}_g'].astype(np.float64)
        bb = inputs[f'bn{li}_b'].astype(np.float64)
        m = inputs[f'bn{li}_m'].astype(np.float64)
        v = inputs[f'bn{li}_v'].astype(np.float64)
        sc = g / np.sqrt(v + EPS)
        return (cw * sc[:, None]).T, (cb - m) * sc + bb

    W1, b1 = fold(1)
    W2, b2 = fold(2)
    W3, b3 = fold(3)
    W4 = inputs['conv4_w'].T.astype(np.float64)
    b4 = inputs['conv4_b'].astype(np.float64)
    w5 = inputs['conv5_w'][0].astype(np.float64)
    b5 = float(inputs['conv5_b'][0])
    Ws = W4 @ w5
    bs = float(b4 @ w5 + b5)

    f8 = ml_dtypes.float8_e4m3
    bf = ml_dtypes.bfloat16
    w['w1d'] = W1.astype(np.float32).reshape(128, 2, 128).astype(f8)
    w['w2'] = W2.astype(np.float32).astype(bf)
    w['w3'] = W3.astype(np.float32).astype(bf)
    wsc2 = np.stack([Ws, -Ws], axis=1).astype(np.float32)      # [32, 2]
    w['wsc2'] = wsc2.astype(bf)
    w['b1'] = b1.reshape(128, 1).astype(np.float32)
    w['b2'] = b2.reshape(64, 1).astype(np.float32)
    w['b3'] = b3.reshape(32, 1).astype(np.float32)

    w['rw1'] = W1.astype(np.float32).reshape(2, 128, 128).transpose(1, 0, 2).copy()
    w['rw2'] = W2.astype(np.float32).copy()
    w['rw3'] = W3.astype(np.float32).copy()
    w['rw45'] = np.concatenate([W4, Ws.reshape(32, 1)], axis=1).astype(np.float32)
    w['rb45'] = np.concatenate([b4, [bs]]).reshape(9, 1).astype(np.float32)

    sq = 1.0 / math.sqrt(D)
    qw = inputs['q_w'].astype(np.float32) * sq
    qb = inputs['q_b'].astype(np.float32) * sq
    w['qw'] = np.concatenate([qw.T, qb.reshape(1, 8)], axis=0).astype(np.float32)
    w['kw'] = np.concatenate([inputs['k_w'].T, inputs['k_b'].reshape(1, 8)],
                             axis=0).astype(np.float32)
    w['vw'] = np.concatenate([inputs['v_w'].T, inputs['v_b'].reshape(1, 8)],
                             axis=0).astype(np.float32)
    w['w6'] = (ALPHA * inputs['conv6_w'].astype(np.float32)).reshape(1, 8)
    w['b6'] = (ALPHA * inputs['conv6_b'].astype(np.float32)).reshape(1, 8)

    ident = np.eye(128, dtype=np.float32)
    w['ident'] = ident
    w['ident16'] = ident.astype(bf)
    onesmask = np.zeros((128, 2), np.float32)
    onesmask[:, 0] = 1.0
    onesmask[:72, 1] = 1.0
    w['onesmask'] = onesmask
    w['c512'] = np.array([[0.0], [512.0]], np.float32)
    w['sgn'] = np.array([[1.0], [-1.0]], np.float32)
    # selection constants
    P = np.arange(SP)
    w['cA'] = (3584.0 * (P // 4) + 7.0 * (P % 4)).reshape(SP, 1).astype(np.float32)
    w['iota16'] = np.tile(np.arange(16, dtype=np.float32), (SP, 1))
    w['iota512'] = np.tile(np.arange(CAP, dtype=np.float32), (2, 1))
    lpre = (P[:, None] < P[None, :]).astype(np.float32)        # [K, M] K<M
    w['lpre'] = lpre
    w['ones112'] = np.ones((SP, 1), np.float32)
    w['zcap'] = np.zeros((8, 2 * CAP // 8), np.uint32)
    w['negc'] = np.full((1, 336), NEGP, np.float32)
    return w


WEIGHT_SPECS = [
    ('w1d', [128, 2, 128], F8),
    ('w2', [128, 64], BF16), ('w3', [64, 32], BF16), ('wsc2', [32, 2], BF16),
    ('b1', [128, 1], F32), ('b2', [64, 1], F32), ('b3', [32, 1], F32),
    ('rw1', [128, 2, 128], F32), ('rw2', [128, 64], F32), ('rw3', [64, 32], F32),
    ('rw45', [32, 9], F32), ('rb45', [9, 1], F32),
    ('qw', [9, 8], F32), ('kw', [9, 8], F32), ('vw', [9, 8], F32),
    ('w6', [1, 8], F32), ('b6', [1, 8], F32),
    ('ident', [128, 128], F32), ('ident16', [128, 128], BF16),
    ('onesmask', [128, 2], F32), ('c512', [2, 1], F32), ('sgn', [2, 1], F32),
    ('cA', [SP, 1], F32), ('iota16', [SP, 16], F32), ('iota512', [2, CAP], F32),
    ('lpre', [SP, SP], F32), ('ones112', [SP, 1], F32),
    ('zcap', [8, 2 * CAP // 8], U32), ('negc', [1, 336], F32),
]


def build_bass():
    nc = bacc.Bacc("TRN2", target_bir_lowering=False, debug=False)

    x_d = nc.dram_tensor("x", [N, C], F32, kind="ExternalInput")
    wd = {}
    for name, shape, dt in WEIGHT_SPECS:
        wd[name] = nc.dram_tensor(name, shape, dt, kind="ExternalInput")
    out_d = nc.dram_tensor("out", [1, D], F32, kind="ExternalOutput")

    compact_d = nc.dram_tensor("compact_scratch", [2 * CAP, 1], U32, kind="Internal")
    cand_d = nc.dram_tensor("cand_scratch", [2 * CAP, 16], F32, kind="Internal")
    sc_d = nc.dram_tensor("sc_scratch", [2 * CAP], F32, kind="Internal")
    vi_d = nc.dram_tensor("vi_scratch", [208], U32, kind="Internal")
    vv_d = nc.dram_tensor("vv_scratch", [208], F32, kind="Internal")
    wb_d = nc.dram_tensor("w_scratch", [256], F32, kind="Internal")

    with TileContext(nc) as tc:
        with tc.tile_pool(name="consts", bufs=1) as cpool:
            ws = {}
            for name, shape, dt in WEIGHT_SPECS:
                t = cpool.tile(shape, dt, tag=name)
                sl = t[0:shape[0]]
                nc.sync.dma_start(sl, wd[name].ap())
                ws[name] = sl

            # prefill compact scratch with zeros (pad rows gather row 0)
            pre_i = nc.sync.dma_start(
                compact_d.ap().rearrange("(p f) o -> p (f o)", p=8), ws['zcap'])

            # persistent selection tiles
            sel = {}
            for nm, shape, dt in [
                ('s_all', [SP, 2, 896], F32),
                ('v1p', [SP, 16], F32), ('v1n', [SP, 16], F32),
                ('i1p', [SP, 16], U32), ('i1n', [SP, 16], U32),
                ('mxp', [SP, 1], F32), ('mxn', [SP, 1], F32),
                ('posp', [SP, 16], F32), ('posn', [SP, 16], F32),
            ]:
                t = cpool.tile(shape, dt, tag=nm)
                sel[nm] = t[0:shape[0]]

            def decode_pos(i1, posf, tmp_pool, p0, p1):
                """pos = cA + 28*(i%128) + i//128 for col-index i1[p0:p1]."""
                sl = slice(p0, p1)
                a_t = tmp_pool.tile([SP, 16], F32, tag="dec_a")
                b_t = tmp_pool.tile([SP, 16], F32, tag="dec_b")
                bu_t = tmp_pool.tile([SP, 16], U32, tag="dec_u")
                a, b, bu = a_t[sl], b_t[sl], bu_t[sl]
                nc.vector.tensor_copy(a[:], i1[sl])                      # i as f32
                nc.vector.tensor_scalar(b[:], a[:], 0.5, 1.0 / 128.0,
                                        op0=ALU.add, op1=ALU.mult)
                nc.vector.tensor_copy(bu[:], b[:])                       # r = i//128
                nc.vector.tensor_copy(b[:], bu[:])
                nc.vector.tensor_scalar(posf[sl], b[:], -128.0, None, op0=ALU.mult)
                nc.vector.tensor_add(posf[sl], posf[sl], a[:])           # p = i-128r
                nc.vector.tensor_scalar(posf[sl], posf[sl], 28.0, None, op0=ALU.mult)
                nc.vector.tensor_add(posf[sl], posf[sl], b[:])           # 28p + r
                nc.vector.tensor_scalar(posf[sl], posf[sl], ws['cA'][sl], None,
                                        op0=ALU.add)

            def half_filter(h, tmp_pool):
                """per-partition top-16 + indices + maxes for partitions of half h."""
                p0, p1 = (0, 64) if h == 0 else (64, SP)
                sl = slice(p0, p1)
                for sg_i, v1, i1, mx, posf in (
                        (0, sel['v1p'], sel['i1p'], sel['mxp'], sel['posp']),
                        (1, sel['v1n'], sel['i1n'], sel['mxn'], sel['posn'])):
                    s_v = sel['s_all'][sl, sg_i]
                    nc.vector.reduce_max(mx[sl], s_v, axis=AX.X)
                    nc.vector.max(out=v1[sl, 0:8], in_=s_v)
                    nc.vector.max_index(out=i1[sl, 0:8], in_max=v1[sl, 0:8],
                                        in_values=s_v)
                    nc.vector.match_replace(out=s_v, in_to_replace=v1[sl, 0:8],
                                            in_values=s_v, imm_value=NEGR)
                    nc.vector.max(out=v1[sl, 8:16], in_=s_v)
                    nc.vector.max_index(out=i1[sl, 8:16], in_max=v1[sl, 8:16],
                                        in_values=s_v)
                for i1, posf in ((sel['i1p'], sel['posp']), (sel['i1n'], sel['posn'])):
                    decode_pos(i1, posf, tmp_pool, p0, p1)

            # ---------------- Phase A ----------------
            with (
                tc.tile_pool(name="xin", bufs=3) as xpool,
                tc.tile_pool(name="work", bufs=2) as wpool,
                tc.tile_pool(name="hbuf", bufs=2) as hpool,
                tc.tile_pool(name="stg", bufs=2) as stpool,
                tc.tile_pool(name="ps_xt", bufs=2, space="PSUM") as ps_xt,
                tc.tile_pool(name="ps_l1", bufs=2, space="PSUM") as ps_l1,
                tc.tile_pool(name="ps_l23", bufs=2, space="PSUM") as ps_l23,
                tc.tile_pool(name="ps_s", bufs=1, space="PSUM") as ps_sp,
            ):
                stage_dmas = {}
                for g in range(NG):
                    r0 = g * GRP
                    x_sb = xpool.tile([128, 28, C], F8, tag="x")
                    if g < NG - 1:
                        nc.gpsimd.dma_start(
                            x_sb[:],
                            x_d.ap()[r0:r0 + GRP, :].rearrange(
                                "(p r) c -> p (r c)", p=128),
                        )
                    else:
                        nc.vector.memset(x_sb[:], 0.0)
                        # rows 96768..99988 -> partitions 0..114 (28 each)
                        nc.gpsimd.dma_start(
                            x_sb[0:115],
                            x_d.ap()[r0:r0 + 115 * 28, :].rearrange(
                                "(p r) c -> p (r c)", p=115),
                        )
                        # rows 99988..100000 -> partition 115, rows 0..12
                        nc.gpsimd.dma_start(
                            x_sb[115:116, 0:12],
                            x_d.ap()[r0 + 115 * 28:N, :].rearrange(
                                "(p r) c -> p (r c)", p=1),
                        )

                    xT = wpool.tile([128, 4, 896], BF16, tag="xT")
                    ps_score = ps_sp.tile([128, 1024], F32, tag="pss")
                    nc.vector.memset(ps_score[:], 0.0)
                    for half in range(4):
                        pst = ps_xt.tile([128, 896], BF16, tag="psxT")
                        for kk in range(7):
                            k = half * 7 + kk
                            nc.tensor.transpose(
                                pst[:, kk * 128:(kk + 1) * 128],
                                x_sb[:, k].bitcast(BF16),
                                ws['ident16'],
                            )
                        nc.vector.tensor_copy(xT[:, half], pst[:])

                        for cc in range(2):
                            c = half * 2 + cc
                            # L1 fp8 DoubleRow over pair-packed xT
                            ph1 = ps_l1.tile([128, 448], F32, tag="ps1")
                            rhs = xT[:, half].bitcast(F8).rearrange(
                                "p (n two) -> p two n", two=2)[:, :, cc * 448:(cc + 1) * 448]
                            nc.tensor.matmul(ph1[:], lhsT=ws['w1d'], rhs=rhs,
                                             start=True, stop=True, perf_mode=DR)
                            h1 = hpool.tile([128, 448], BF16, tag="h1")
                            nc.scalar.activation(h1[:], ph1[:], AF.Relu, bias=ws['b1'])

                            ph2 = ps_l23.tile([64, 448], F32, tag="ps23")
                            nc.tensor.matmul(ph2[:], lhsT=ws['w2'], rhs=h1[:],
                                             start=True, stop=True)
                            h2 = hpool.tile([64, 448], BF16, tag="h2")
                            nc.scalar.activation(h2[:], ph2[:], AF.Relu, bias=ws['b2'])

                            ph3 = ps_l23.tile([32, 448], F32, tag="ps23")
                            nc.tensor.matmul(ph3[:], lhsT=ws['w3'], rhs=h2[:],
                                             start=True, stop=True)
                            h3 = hpool.tile([32, 448], BF16, tag="h3")
                            nc.vector.tensor_scalar(h3[:], ph3[:], ws['b3'], 0.0,
                                                    op0=ALU.add, op1=ALU.max)

                            pb_ = 32 * (c // 2)
                            nc.tensor.matmul(
                                ps_score[pb_:pb_ + 2,
                                         512 * (c % 2):512 * (c % 2) + 448],
                                lhsT=ws['wsc2'], rhs=h3[:],
                                start=True, stop=True,
                                tile_position=(0, pb_),
                            )

                    st_sb = stpool.tile([98, 1024], F32, tag="st")
                    cp_i = nc.vector.tensor_copy(st_sb[:], ps_score[0:98])
                    if g >= 2:
                        for d_ in stage_dmas[g - 2]:
                            add_dep_helper(cp_i.ins, d_.ins,
                                           reason="stage reuse after dma")
                    stage_dmas[g] = []
                    for a in range(4):
                        d_i = nc.sync.dma_start(
                            sel['s_all'][4 * g + a:4 * g + a + 1],
                            st_sb[32 * a:32 * a + 2].rearrange(
                                "s (b i) -> s b i", b=2)[:, :, 0:448],
                        )
                        add_dep_helper(d_i.ins, cp_i.ins, reason="stage dma after copy")
                        stage_dmas[g].append(d_i)

                    if g == 15:
                        half_filter(0, wpool)
                    if g == NG - 1:
                        # pad scores -> NEGP (positions >= N), via DMA
                        for sg_i in range(2):
                            nc.sync.dma_start(
                                sel['s_all'][108:112, sg_i].rearrange(
                                    "q (r p) -> q r p", r=7)[:, :, 116:128],
                                ws['negc'][:, 0:336])
                            nc.sync.dma_start(
                                sel['s_all'][109:110, sg_i].rearrange(
                                    "o (r p) -> o r p", r=7)[:, 5:7, 115:116],
                                ws['negc'][:, 0:2])
                            nc.sync.dma_start(
                                sel['s_all'][110:112, sg_i].rearrange(
                                    "q (r p) -> q r p", r=7)[:, :, 115:116],
                                ws['negc'][:, 0:14])
                        half_filter(1, wpool)

            # ---------------- Selection tail + Phase B ----------------
            with (
                tc.tile_pool(name="pb", bufs=1) as pb,
                tc.tile_pool(name="ps_b", bufs=1, space="PSUM") as psb,
            ):
                # theta = 104th-largest of maxes = 9th-smallest
                mx12 = pb.tile([SP, 2], F32)
                nc.vector.tensor_copy(mx12[:, 0:1], sel['mxp'][:])
                nc.vector.tensor_copy(mx12[:, 1:2], sel['mxn'][:])
                nc.vector.tensor_scalar_mul(mx12[:], mx12[:], -1.0)
                psq = psb.tile([2, SP], F32, tag="psb512")
                nc.tensor.transpose(psq[:], mx12[:], ws['ident'][0:SP, 0:SP])
                nmx = pb.tile([2, SP], F32)
                nc.vector.tensor_copy(nmx[:], psq[:])
                b8a = pb.tile([2, 8], F32)
                nc.vector.max(out=b8a[:], in_=nmx[:])
                nc.vector.match_replace(out=nmx[:], in_to_replace=b8a[:],
                                        in_values=nmx[:], imm_value=NEG)
                b8b = pb.tile([2, 8], F32)
                nc.vector.max(out=b8b[:], in_=nmx[:])
                psq2 = psb.tile([8, 2], F32, tag="psb512")
                nc.tensor.transpose(psq2[:], b8b[:], ws['ident'][0:2, 0:2])
                thT = pb.tile([8, 2], F32)
                nc.vector.tensor_copy(thT[:], psq2[:])
                # thT[0, :] = (-theta_s, -theta_n)
                thb = pb.tile([SP, 2], F32)
                nc.gpsimd.partition_broadcast(thb[:], thT[0:1, :])

                # survivors: cmp = (v1 + (-theta)) >= 0
                cmpp = pb.tile([SP, 16], F32)
                cmpn = pb.tile([SP, 16], F32)
                nc.vector.tensor_scalar(cmpp[:], sel['v1p'][:], thb[:, 0:1], 0.0,
                                        op0=ALU.add, op1=ALU.is_ge)
                nc.vector.tensor_scalar(cmpn[:], sel['v1n'][:], thb[:, 1:2], 0.0,
                                        op0=ALU.add, op1=ALU.is_ge)
                noth = pb.tile([SP, 2], F32)
                nc.vector.tensor_reduce(noth[:, 0:1], cmpp[:], axis=AX.X, op=ALU.add)
                nc.vector.tensor_reduce(noth[:, 1:2], cmpn[:], axis=AX.X, op=ALU.add)

                ps_off = psb.tile([SP, 2], F32, tag="psb512")
                nc.tensor.matmul(ps_off[:], lhsT=ws['lpre'], rhs=noth[:],
                                 start=True, stop=True)
                offc2 = pb.tile([SP, 2], F32)
                nc.vector.tensor_copy(offc2[:], ps_off[:])
                ps_n = psb.tile([2, 1], F32, tag="psb512")
                nc.tensor.matmul(ps_n[:], lhsT=noth[:], rhs=ws['ones112'],
                                 start=True, stop=True)
                n2 = pb.tile([2, 1], F32)
                nc.vector.tensor_copy(n2[:], ps_n[:])

                # offsets with OOB pads: off + k + 2048*(1-cmp)  (+512 for neg)
                scat = []
                for cmpx, offcol, base, posf in (
                        (cmpp, 0, 0.0, sel['posp']), (cmpn, 1, 512.0, sel['posn'])):
                    basei = pb.tile([SP, 16], F32, tag="sc_base")
                    nc.vector.tensor_scalar(basei[:], ws['iota16'], offc2[:, offcol:offcol + 1],
                                            base, op0=ALU.add, op1=ALU.add)
                    pen = pb.tile([SP, 16], F32, tag="sc_pen")
                    nc.vector.tensor_scalar(pen[:], cmpx[:], -2048.0, 2048.0,
                                            op0=ALU.mult, op1=ALU.add)
                    nc.vector.tensor_add(basei[:], basei[:], pen[:])
                    offu = pb.tile([SP, 16], U32, tag="sc_offu")
                    nc.vector.tensor_copy(offu[:], basei[:])
                    posu = pb.tile([SP, 16], U32, tag="sc_posu")
                    nc.vector.tensor_copy(posu[:], posf[:])
                    s_i = nc.gpsimd.indirect_dma_start(
                        out=compact_d.ap(),
                        out_offset=bass.IndirectOffsetOnAxis(ap=offu[:], axis=0),
                        in_=posu[:], in_offset=None,
                        bounds_check=2 * CAP - 1, oob_is_err=False,
                    )
                    add_dep_helper(s_i.ins, pre_i.ins, reason="scatter after prefill")
                    scat.append(s_i)

                idxg = pb.tile([128, 8], U32)
                ir_ = nc.sync.dma_start(
                    idxg[:], compact_d.ap().rearrange("(k p) o -> p k o", p=128))
                for s_i in scat:
                    add_dep_helper(ir_.ins, s_i.ins, reason="idx read after scatter")

                # gather candidate x rows [128, 8, 256]
                xg = pb.tile([128, 8, C], F32)
                xg_g = []
                for k in range(8):
                    g_i = nc.gpsimd.indirect_dma_start(
                        out=xg[:, k], out_offset=None,
                        in_=x_d.ap(),
                        in_offset=bass.IndirectOffsetOnAxis(ap=idxg[:, k:k + 1], axis=0),
                    )
                    xg_g.append(g_i)

                # transpose candidates to channel-major
                xgT = pb.tile([128, 2, 2 * CAP], F32)
                for ch in range(2):
                    for cc in range(2):
                        pst = psb.tile([128, 512], F32, tag="psb512")
                        for tq in range(4):
                            tcol = ch * 4 + tq
                            tr_i = nc.tensor.transpose(
                                pst[:, tq * 128:(tq + 1) * 128],
                                xg[:, tcol, cc * 128:(cc + 1) * 128],
                                ws['ident'],
                            )
                            add_dep_helper(tr_i.ins, xg_g[tcol].ins,
                                           reason="transpose after gather")
                        nc.vector.tensor_copy(
                            xgT[:, cc, ch * 512:(ch + 1) * 512], pst[:])

                # exact refine MLP (f32)
                r45 = pb.tile([128, 2 * CAP], F32)
                nc.vector.memset(r45[:], 0.0)
                for ch in range(2):
                    sl = slice(ch * 512, (ch + 1) * 512)
                    ps1 = psb.tile([128, 512], F32, tag="psb512")
                    for cc in range(2):
                        nc.tensor.matmul(ps1[:], lhsT=ws['rw1'][:, cc],
                                         rhs=xgT[:, cc, sl],
                                         start=(cc == 0), stop=(cc == 1))
                    rh1 = pb.tile([128, 512], F32, tag="rh1")
                    nc.scalar.activation(rh1[:], ps1[:], AF.Relu, bias=ws['b1'])
                    ps2 = psb.tile([64, 512], F32, tag="psb512")
                    nc.tensor.matmul(ps2[:], lhsT=ws['rw2'], rhs=rh1[:],
                                     start=True, stop=True)
                    rh2 = pb.tile([64, 512], F32, tag="rh2")
                    nc.scalar.activation(rh2[:], ps2[:], AF.Relu, bias=ws['b2'])
                    ps3 = psb.tile([32, 512], F32, tag="psb512")
                    nc.tensor.matmul(ps3[:], lhsT=ws['rw3'], rhs=rh2[:],
                                     start=True, stop=True)
                    rh3 = pb.tile([32, 512], F32, tag="rh3")
                    nc.scalar.activation(rh3[:], ps3[:], AF.Relu, bias=ws['b3'])
                    ps4 = psb.tile([9, 512], F32, tag="psb512")
                    nc.tensor.matmul(ps4[:], lhsT=ws['rw45'], rhs=rh3[:],
                                     start=True, stop=True)
                    nc.vector.tensor_scalar(r45[0:9, sl], ps4[:], ws['rb45'],
                                            None, op0=ALU.add)

                # store candidate rows [1024, 16] for the final gather
                candT = pb.tile([128, 8, 16], F32)
                nc.vector.memset(candT[:], 0.0)
                for blk in range(8):
                    pst45 = psb.tile([128, 128], F32, tag="psb512")
                    nc.tensor.transpose(pst45[:],
                                        r45[:, blk * 128:(blk + 1) * 128],
                                        ws['ident'])
                    nc.vector.tensor_copy(candT[:, blk, 0:9], pst45[:, 0:9])
                candw = nc.sync.dma_start(
                    cand_d.ap().rearrange("(t p) f -> p t f", p=128), candT[:])

                # final exact select on [2, 512]
                selw = pb.tile([2, CAP], F32)
                scw = nc.sync.dma_start(sc_d.ap().rearrange("(o f) -> o f", o=1),
                                        r45[8:9, :])
                scr = nc.sync.dma_start(selw[:],
                                        sc_d.ap().rearrange("(t c) -> t c", t=2))
                add_dep_helper(scr.ins, scw.ins, reason="sc bounce order")
                nc.vector.tensor_scalar(selw[:], selw[:], ws['sgn'], None, op0=ALU.mult)
                cmpM = pb.tile([2, CAP], F32)
                nc.vector.tensor_scalar(cmpM[:], ws['iota512'], n2[:], None,
                                        op0=ALU.is_lt)
                nc.vector.tensor_mul(selw[:], selw[:], cmpM[:])
                nc.vector.tensor_scalar(cmpM[:], cmpM[:], -NEGR, NEGR,
                                        op0=ALU.mult, op1=ALU.add)
                nc.vector.tensor_add(selw[:], selw[:], cmpM[:])

                vals = pb.tile([2, 104], F32)
                cidx = pb.tile([2, 104], U32)
                for r_ in range(13):
                    sl = slice(r_ * 8, (r_ + 1) * 8)
                    nc.vector.max(out=vals[:, sl], in_=selw[:])
                    nc.vector.max_index(out=cidx[:, sl], in_max=vals[:, sl],
                                        in_values=selw[:])
                    nc.vector.match_replace(out=selw[:], in_to_replace=vals[:, sl],
                                            in_values=selw[:], imm_value=NEG)

                # candidate row ids; un-negate lo values
                cidx_f = pb.tile([2, 104], F32)
                nc.vector.tensor_copy(cidx_f[:], cidx[:])
                nc.vector.tensor_scalar(cidx_f[:], cidx_f[:], ws['c512'], None,
                                        op0=ALU.add)
                ccol = pb.tile([2, 104], U32)
                nc.vector.tensor_copy(ccol[:], cidx_f[:])
                nc.vector.tensor_scalar(vals[:], vals[:], ws['sgn'], None, op0=ALU.mult)

                # token-order index [128, 2] and values [128, 2]
                ccol_g = pb.tile([128, 2], U32)
                vals_g = pb.tile([128, 2], F32)
                for (dst, src_t, bd) in ((ccol_g, ccol, vi_d), (vals_g, vals, vv_d)):
                    bw_ = nc.sync.dma_start(bd.ap().rearrange("(p f) -> p f", p=2),
                                            src_t[:])
                    bda = bd.ap()
                    for (osl, isl) in (((slice(0, 100), 0), (0, 100)),
                                       ((slice(100, 128), 0), (104, 132)),
                                       ((slice(0, 72), 1), (132, 204)),
                                       ((slice(72, 128), 1), (148, 204))):
                        r_i = nc.sync.dma_start(
                            dst[osl[0], osl[1]:osl[1] + 1],
                            bda[isl[0]:isl[1]].rearrange("f -> f ()"))
                        add_dep_helper(r_i.ins, bw_.ins, reason="vi/vv bounce order")

                x4a = pb.tile([128, 2, 16], F32)
                x4a_g = []
                for tcol in range(2):
                    g_i = nc.gpsimd.indirect_dma_start(
                        out=x4a[:, tcol], out_offset=None,
                        in_=cand_d.ap(),
                        in_offset=bass.IndirectOffsetOnAxis(
                            ap=ccol_g[:, tcol:tcol + 1], axis=0),
                    )
                    add_dep_helper(g_i.ins, candw.ins, reason="gather after cand write")
                    x4a_g.append(g_i)

                w6b = pb.tile([128, 8], F32)
                nc.gpsimd.partition_broadcast(w6b[:], ws['w6'])
                b6b = pb.tile([128, 8], F32)
                nc.gpsimd.partition_broadcast(b6b[:], ws['b6'])

                h2a = pb.tile([128, 2, 9], F32)
                for tcol in range(2):
                    nc.vector.tensor_scalar(h2a[:, tcol, 0:8], w6b[:],
                                            vals_g[:, tcol:tcol + 1], None,
                                            op0=ALU.mult)
                    a_i = nc.vector.tensor_add(h2a[:, tcol, 0:8], h2a[:, tcol, 0:8],
                                               x4a[:, tcol, 0:8])
                    add_dep_helper(a_i.ins, x4a_g[tcol].ins,
                                   reason="h2 after x4a gather")
                    nc.vector.tensor_add(h2a[:, tcol, 0:8], h2a[:, tcol, 0:8], b6b[:])
                nc.vector.memset(h2a[:, :, 8:9], 1.0)

                h2T = pb.tile([9, TOK], F32)
                psh = psb.tile([16, 256], F32, tag="psb_h2t")
                for tcol in range(2):
                    nc.tensor.transpose(
                        psh[0:9, tcol * 128:(tcol + 1) * 128],
                        h2a[:, tcol], ws['ident'])
                nc.vector.tensor_copy(h2T[:], psh[0:9, :])

                psq_ = psb.tile([8, TOK], F32, tag="psb_q")
                nc.tensor.matmul(psq_[:], lhsT=ws['qw'], rhs=h2T[:], start=True,
                                 stop=True)
                qT = pb.tile([8, TOK], F32)
                nc.vector.tensor_copy(qT[:], psq_[:])
                psk = psb.tile([8, TOK], F32, tag="psb_q")
                nc.tensor.matmul(psk[:], lhsT=ws['kw'], rhs=h2T[:], start=True,
                                 stop=True)
                kT = pb.tile([8, TOK], F32)
                nc.vector.tensor_copy(kT[:], psk[:])

                v_sb = pb.tile([128, 2, 8], F32)
                for tcol in range(2):
                    psv = psb.tile([128, 8], F32, tag="psb_v")
                    nc.tensor.matmul(psv[:], lhsT=h2T[:, tcol * 128:(tcol + 1) * 128],
                                     rhs=ws['vw'], start=True, stop=True)
                    nc.vector.tensor_copy(v_sb[:, tcol], psv[:])

                psw = psb.tile([1, TOK], F32, tag="psb_w")
                for tcol in range(2):
                    psS = psb.tile([128, TOK], F32, tag="psb_S")
                    nc.tensor.matmul(psS[:], lhsT=qT[:, tcol * 128:(tcol + 1) * 128],
                                     rhs=kT[:], start=True, stop=True)
                    nc.vector.memset(psS[:, 200:], NEG)
                    mrow = pb.tile([128, 1], F32, tag="mrow")
                    nc.vector.reduce_max(mrow[:], psS[:], axis=AX.X)
                    mneg = pb.tile([128, 1], F32, tag="mneg")
                    nc.vector.tensor_scalar_mul(mneg[:], mrow[:], -1.0)
                    pexp = pb.tile([128, TOK], F32, tag="pexp")
                    sume = pb.tile([128, 1], F32, tag="sume")
                    nc.scalar.activation(pexp[:], psS[:], AF.Exp, bias=mneg[:],
                                         accum_out=sume[:])
                    rsum = pb.tile([128, 1], F32, tag="rsum")
                    nc.vector.reciprocal(rsum[:], sume[:])
                    nc.vector.tensor_mul(rsum[:], rsum[:],
                                         ws['onesmask'][:, tcol:tcol + 1])
                    nc.tensor.matmul(psw[:], lhsT=rsum[:], rhs=pexp[:],
                                     start=(tcol == 0), stop=(tcol == 1))

                w_sb = pb.tile([1, TOK], F32)
                nc.vector.tensor_copy(w_sb[:], psw[:])
                wbw = nc.sync.dma_start(wb_d.ap().rearrange("(o f) -> o f", o=1),
                                        w_sb[:])
                wT = pb.tile([128, 2], F32)
                wbr = nc.sync.dma_start(wT[:],
                                        wb_d.ap().rearrange("(t p) -> p t", p=128))
                add_dep_helper(wbr.ins, wbw.ins, reason="w bounce order")

                psp = psb.tile([1, 8], F32, tag="psb_p")
                for tcol in range(2):
                    nc.tensor.matmul(psp[:], lhsT=wT[:, tcol:tcol + 1],
                                     rhs=v_sb[:, tcol], start=(tcol == 0),
                                     stop=(tcol == 1))

                mm = pb.tile([1, 1], F32)
                nc.vector.reduce_max(mm[:], psp[:], axis=AX.X)
                mneg8 = pb.tile([1, 1], F32)
                nc.vector.tensor_scalar_mul(mneg8[:], mm[:], -1.0 / 200.0)
                e8 = pb.tile([1, 8], F32)
                s8 = pb.tile([1, 1], F32)
                nc.scalar.activation(e8[:], psp[:], AF.Exp, bias=mneg8[:],
                                     scale=1.0 / 200.0, accum_out=s8[:])
                r8 = pb.tile([1, 1], F32)
                nc.vector.reciprocal(r8[:], s8[:])
                outv = pb.tile([1, 8], F32)
                nc.vector.tensor_scalar(outv[:], e8[:], r8[:], None, op0=ALU.mult)
                nc.sync.dma_start(out_d.ap(), outv[:])

    nc.compile()
    return nc


def kernel(**inputs):
    key = 'nc'
    if key not in _CACHE:
        _CACHE[key] = build_bass()
    nc = _CACHE[key]

    w = _fold_weights(inputs)
    x = np.ascontiguousarray(np.asarray(inputs['x'], dtype=np.float32))
    in_maps = []
    for b in range(B):
        m = {'x': x[b]}
        for name, shape, dt in WEIGHT_SPECS:
            m[name] = w[name]
        in_maps.append(m)

    res = bass_utils.run_bass_kernel_spmd(nc, in_maps, core_ids=list(range(B)))
    LAST['res'] = res
    out = np.stack([res.results[b]['out'][0] for b in range(B)], axis=0)
    return out.astype(np.float32)


if __name__ == '__main__':
    nc = build_bass()
    print("build ok")
